# revision 20
# baseline (speedup 1.0000x reference)
"""Trainium2 Bass kernel for nn_DotProductAttention_11433202942822.

Math (per (b, h) pair, T=2048, D=64):
    S = Q @ K^T * (1/sqrt(64))            [T1, T2]
    attn = softmax(S, axis=T1)            <- softmax over the QUERY axis
    out = attn @ V                        [T1, D]

Structure (v4):
  * S^T = K @ Q^T per 128-row k2-tile, with the two q-halves of each tile
    computed CONCURRENTLY in the two PE row-group halves (contraction is
    d=64, so rows 0-63 and 64-127 hold two independent copies of the
    weights; Q^T/K^T are duplicated across both partition halves).
  * The exp+colsum drain of the PSUM score tiles is split across TWO
    engines per-tile ("routing"):
      - ACT route: scalar-engine activation(Exp, accum_out) -> fp16 et
        plus free column sums via the ACT accumulator.
      - DVE route: one-pass Schraudolph exp on the vector engine:
        int16 bits = trunc(s * A + B) computed by tensor_scalar
        (mult+add, fp32 PSUM -> int16 SBUF), bit-identical to fp16
        exp2 values; the int16 tile is bitcast to fp16 for mm2. The
        column sums are a GPSIMD 2-level pairwise-add tree + one small
        DVE reduce, so the vector engine only pays ~1/4 of a reduce.
  * Softmax normalization is folded into V (vp = V * 1/colsum), batched
    per half-head with a broadcast tensor_tensor.
  * mm2 (out^T += vp^T @ et) packs the 64-wide output into both PE
    column-group halves for 2x concurrency (as row-pairing does for mm1).

Sharding: batch*heads = 32 pairs, 4 per core across 8 cores (head/data
parallel, no cross-core communication).
"""

import sys

import numpy as np

if "/opt/trn_rl_repo" not in sys.path:
    sys.path.insert(0, "/opt/trn_rl_repo")

import concourse.tile as tile  # noqa: E402
from concourse import bacc, mybir  # noqa: E402
from concourse.bass_utils import run_bass_kernel_spmd  # noqa: E402

P = 128
D = 64
SCALE = 1.0 / (D ** 0.5)
N_CORES = 8

F32 = mybir.dt.float32
F16 = mybir.dt.float16
I16 = mybir.dt.int16

# Schraudolph fp16 exp: bits16 = trunc(s * A_TS + B_TS) interpreted as fp16
# gives exp2(s*log2e*SCALE + c') = exp(s*SCALE)*2^c. The uniform 2^c factor
# cancels in the softmax normalization; c is tuned for min rel-rms anyway.
LOG2E = 1.4426950408889634
A_TS = SCALE * LOG2E * 1024.0
B_TS = (15.0 - 0.0575) * 1024.0

# Tiles (of 16 per head) drained by the DVE Schraudolph route; the rest
# go to the ACT exp route. Tuned so ACT-busy ~= DVE-busy ~= GPS-busy.
# D-tiles sit early in each half-head so their (longer-latency) colsum
# chain finishes well before the vp boundary.
ROUTE_D = (0, 2, 4, 8, 10, 12)


def build_attention_nc(BH: int, T: int, debug: bool = False):
    """Build the per-core Bass module.

    Inputs (per core):
      qt  [BH, P, T]     fp16  Q^T duplicated across both partition halves
      kt  [BH, P, T]     fp16  K^T duplicated across both partition halves
      v   [BH, P, T/P, D] f32  V with k2 split (tile, partition)
    Output:
      out [BH, D, T]   f32   out transposed (d-major)
    """
    KT = T // P          # k2 tiles per head (16)
    GRP = 8              # tiles per vp/reciprocal batch
    QH = T // 2          # q half (1024)

    nc = bacc.Bacc("TRN2", target_bir_lowering=False, debug=debug)

    qt = nc.dram_tensor("qt", [BH, P, T], F16, kind="ExternalInput").ap()
    kt = nc.dram_tensor("kt", [BH, P, T], F16, kind="ExternalInput").ap()
    v = nc.dram_tensor("v", [BH, P, KT, D], F32, kind="ExternalInput").ap()
    out = nc.dram_tensor("out", [BH, D, T], F32, kind="ExternalOutput").ap()

    with tile.TileContext(nc) as tc:
        with (
            tc.tile_pool(name="ins", bufs=1) as ins_pool,
            tc.tile_pool(name="et", bufs=22) as et_pool,
            tc.tile_pool(name="gh", bufs=3) as gh_pool,
            tc.tile_pool(name="small", bufs=4) as small_pool,
            tc.tile_pool(name="vps", bufs=4) as vp_pool,
            tc.tile_pool(name="osb", bufs=2) as osb_pool,
            tc.tile_pool(name="spsum", bufs=1, space="PSUM") as s_pool,
            tc.tile_pool(name="opsum", bufs=1, space="PSUM") as o_pool,
        ):
            qt_sb = ins_pool.tile([P, BH, T], F16, tag="qt_sb")
            kt_sb = ins_pool.tile([P, BH, T], F16, tag="kt_sb")
            v_sb = ins_pool.tile([P, BH, KT, D], F32, tag="v_sb")
            zf = ins_pool.tile([P, 512], F16, tag="zf")
            nc.vector.memset(zf[:], 0.0)
            # Warm up the ACT exp table-set during the input DMAs so the
            # first real ACTIVATE doesn't pay the ~2.6us ACT_TABLE_LOAD.
            warm = small_pool.tile([P, 1], F32, tag="warm")
            nc.vector.memset(warm[:], 0.0)
            nc.scalar.activation(
                warm[:], warm[:], mybir.ActivationFunctionType.Exp
            )

            for bh in range(BH):
                if bh == 0:
                    nc.sync.dma_start(kt_sb[:, bh, :], kt[bh])
                    nc.sync.dma_start(qt_sb[:, bh, 0:QH], qt[bh][:, 0:QH])
                    nc.sync.dma_start(qt_sb[:, bh, QH:T], qt[bh][:, QH:T])
                    nc.sync.dma_start(v_sb[:, bh, 0:KT // 2], v[bh][:, 0:KT // 2])
                    nc.sync.dma_start(v_sb[:, bh, KT // 2:KT], v[bh][:, KT // 2:KT])
                else:
                    nc.sync.dma_start(qt_sb[:, bh, :], qt[bh])
                    nc.sync.dma_start(kt_sb[:, bh, :], kt[bh])
                    nc.sync.dma_start(v_sb[:, bh], v[bh])

            # mm2 jobs are emitted lazily (global across bh) so the PE keeps
            # busy while ACT/DVE drain the next tiles' scores.
            pending_mm2 = []
            mm2_started = set()

            def emit_fillers(out_ps_, n):
                # Mathematically no-op matmuls (both operands zero, start
                # =False accumulate) that keep the PE array streaming during
                # the drain-paced mm1 phase, so the HAM clock-gate doesn't
                # re-throttle the PE to 1.2 GHz between mm2 bursts.
                for _ in range(n):
                    nc.tensor.matmul(
                        out_ps_[0:D, 0:128],
                        lhsT=zf[:, 0:D],
                        rhs=zf[:, 0:128],
                        start=False,
                        stop=False,
                        skip_group_check=True,
                    )

            def emit_mm2(job):
                bh_, t_, out_ps_, vp_, ti_, et_ = job
                mm2_started.add(id(out_ps_))
                lhs = vp_[:, ti_, :]
                for qh in (0, 512):
                    nc.tensor.matmul(
                        out_ps_[0:D, qh:qh + 512],
                        lhsT=lhs,
                        rhs=et_[:, qh:qh + 512],
                        start=(t_ == 0),
                        stop=(t_ == KT - 1),
                        skip_group_check=True,
                    )
                    nc.tensor.matmul(
                        out_ps_[D:2 * D, qh:qh + 512],
                        lhsT=lhs,
                        rhs=et_[:, QH + qh:QH + qh + 512],
                        start=(t_ == 0),
                        stop=(t_ == KT - 1),
                        skip_group_check=True,
                    )
                if t_ == KT - 1:
                    # Evacuate out^T: alternate the copy engine per head to
                    # balance ACT/DVE load, then DMA to DRAM.
                    osb = osb_pool.tile([2 * D, QH], F32, tag="osb")
                    nc.scalar.copy(osb[:], out_ps_[:])
                    nc.sync.dma_start(out[bh_][:, 0:QH], osb[0:D])
                    nc.sync.dma_start(out[bh_][:, QH:T], osb[D:2 * D])

            # Single persistent 6-bank score region; manual 1024-column slot
            # rotation. 2 of every 3 tiles land on a CONTIGUOUS slot pair so
            # their drain is a single FD=2048 instruction (one ACTIVATE +
            # one READ_ACCUMULATOR instead of two of each).
            sreg = s_pool.tile([P, 3 * QH], F32, tag="sreg")
            gseq = 0

            for bh in range(BH):
                out_ps = o_pool.tile([2 * D, QH], F32, tag="out_ps")
                cs0 = small_pool.tile([P, KT], F32, tag="cs0", name="cs0")
                cs1 = small_pool.tile([P, KT], F32, tag="cs1", name="cs1")
                # DVE-route tiles produce a single colsum (into cs0); zero
                # cs1 so the uniform cs0+cs1 add per half-head is correct.
                nc.vector.memset(cs1[:], 0.0)
                et_tiles = {}

                for t in range(KT):
                    sA = (2 * gseq) % 3
                    sB = (2 * gseq + 1) % 3
                    gseq += 1
                    spA = sreg[:, sA * QH:(sA + 1) * QH]
                    spB = sreg[:, sB * QH:(sB + 1) * QH]
                    contig = sB == sA + 1
                    lhsA = kt_sb[0:D, bh, t * P:(t + 1) * P]
                    lhsB = kt_sb[D:2 * D, bh, t * P:(t + 1) * P]
                    # q-halves of tile t run concurrently in the two PE
                    # row-group halves.
                    for c in (0, 512):
                        nc.tensor.matmul(
                            spA[:, c:c + 512],
                            lhsT=lhsA,
                            rhs=qt_sb[0:D, bh, c:c + 512],
                            start=True,
                            stop=True,
                        )
                        nc.tensor.matmul(
                            spB[:, c:c + 512],
                            lhsT=lhsB,
                            rhs=qt_sb[D:2 * D, bh, QH + c:QH + c + 512],
                            start=True,
                            stop=True,
                        )

                    et = et_pool.tile([P, T], F16, tag="et")
                    if t in ROUTE_D:
                        # Schraudolph exp on DVE: int16 bits -> fp16 view.
                        et_i = et[:].bitcast(I16)
                        if contig:
                            nc.vector.tensor_scalar(
                                et_i[:], sreg[:, sA * QH:sA * QH + T],
                                A_TS, B_TS,
                                mybir.AluOpType.mult, mybir.AluOpType.add,
                            )
                        else:
                            nc.vector.tensor_scalar(
                                et_i[:, 0:QH], spA, A_TS, B_TS,
                                mybir.AluOpType.mult, mybir.AluOpType.add,
                            )
                            nc.vector.tensor_scalar(
                                et_i[:, QH:T], spB, A_TS, B_TS,
                                mybir.AluOpType.mult, mybir.AluOpType.add,
                            )
                        # 1-level GPSIMD pairwise add halves the DVE reduce.
                        gh = gh_pool.tile([P, QH], F16, tag="gh")
                        nc.gpsimd.tensor_tensor(
                            gh[:], et[:, 0:QH], et[:, QH:T],
                            mybir.AluOpType.add,
                        )
                        nc.vector.tensor_reduce(
                            cs0[:, t:t + 1], gh[:],
                            mybir.AxisListType.X, mybir.AluOpType.add,
                        )
                    else:
                        if contig:
                            nc.scalar.activation(
                                et[:], sreg[:, sA * QH:sA * QH + T],
                                mybir.ActivationFunctionType.Exp,
                                scale=SCALE,
                                accum_out=cs0[:, t:t + 1],
                            )
                        else:
                            nc.scalar.activation(
                                et[:, 0:QH], spA,
                                mybir.ActivationFunctionType.Exp,
                                scale=SCALE,
                                accum_out=cs0[:, t:t + 1],
                            )
                            nc.scalar.activation(
                                et[:, QH:T], spB,
                                mybir.ActivationFunctionType.Exp,
                                scale=SCALE,
                                accum_out=cs1[:, t:t + 1],
                            )

                    et_tiles[t] = et

                    if t % GRP == GRP - 1:
                        # Group boundary: batched reciprocal + V-prescale.
                        # High priority so the DVE runs this promptly and
                        # mm2 (and thus the PE) isn't starved.
                        h0 = t - (GRP - 1)
                        with tc.high_priority():
                            cst = small_pool.tile([P, GRP], F32, tag="cst")
                            nc.gpsimd.tensor_tensor(
                                cst[:], cs0[:, h0:h0 + GRP],
                                cs1[:, h0:h0 + GRP], mybir.AluOpType.add,
                            )
                            rec = small_pool.tile([P, GRP], F32, tag="rec")
                            nc.vector.reciprocal(rec[:], cst[:])
                            vp = vp_pool.tile([P, GRP, D], F16, tag="vp")
                            nc.vector.tensor_tensor(
                                vp[:],
                                rec[:].unsqueeze(2).broadcast_to([P, GRP, D]),
                                v_sb[:, bh, h0:h0 + GRP, :],
                                mybir.AluOpType.mult,
                            )
                        for ti in range(GRP):
                            pending_mm2.append(
                                (bh, h0 + ti, out_ps, vp, ti,
                                 et_tiles[h0 + ti])
                            )

                    # Emit mm2 in big bursts (8 jobs = 32 MMs ~ 10us of PE
                    # work) so the PE array stays busy long enough for the
                    # HAM clock-gate to unthrottle it to 2.4 GHz.
                    last = bh == BH - 1 and t == KT - 1
                    if last:
                        while pending_mm2:
                            emit_mm2(pending_mm2.pop(0))
                    elif len(pending_mm2) >= GRP:
                        for _ in range(GRP):
                            emit_mm2(pending_mm2.pop(0))
                    elif id(out_ps) in mm2_started:
                        emit_fillers(out_ps, 3)
            while pending_mm2:
                emit_mm2(pending_mm2.pop(0))

    nc.compile()
    return nc


_NC_CACHE: dict = {}

# Debug/profiling knobs (used by the local test harness; harmless defaults).
TRACE = False
LAST_RESULTS = None


def _get_nc(BH: int, T: int):
    key = (BH, T)
    if key not in _NC_CACHE:
        _NC_CACHE[key] = build_attention_nc(BH, T)
    return _NC_CACHE[key]


def _reference_numpy(Q, K, V, padding_mask, isCausal):
    """Fallback exactly mirroring reference.py (never hit for spec inputs)."""
    Q = Q.astype(np.float64)
    K = K.astype(np.float64)
    V = V.astype(np.float64)
    scores = np.einsum("bhqd,bhkd->bhqk", Q, K) * SCALE
    T1 = scores.shape[2]
    mask = padding_mask[:, None, :, :].astype(np.float64)
    if isCausal:
        mask = mask * np.tril(np.ones((T1, T1)))
    scores = np.where(mask == 0, -np.inf, scores)
    m = np.max(scores, axis=2, keepdims=True)
    e = np.exp(scores - m)
    attn = e / np.sum(e, axis=2, keepdims=True)
    return np.einsum("bhqk,bhkd->bhqd", attn, V).astype(np.float32)


def kernel(Q, K, V, padding_mask, isCausal, **_unused):
    Q = np.asarray(Q, dtype=np.float32)
    K = np.asarray(K, dtype=np.float32)
    V = np.asarray(V, dtype=np.float32)
    padding_mask = np.asarray(padding_mask)
    causal = int(np.asarray(isCausal))

    B, H, T, Dd = Q.shape
    assert Dd == D
    if causal != 0 or padding_mask.min() != 1.0 or padding_mask.max() != 1.0:
        return _reference_numpy(Q, K, V, padding_mask, causal)

    BHT = B * H
    assert BHT % N_CORES == 0
    BH = BHT // N_CORES  # pairs per core

    nc = _get_nc(BH, T)

    # Host-side layout prep (contiguous per-core shards).
    Qf = Q.reshape(BHT, T, D)
    Kf = K.reshape(BHT, T, D)
    Vf = V.reshape(BHT, T, D)

    QT = Qf.transpose(0, 2, 1).astype(np.float16)  # [BHT, D, T]
    qt_all = np.ascontiguousarray(
        np.concatenate([QT, QT], axis=1)
    )  # [BHT, 2D, T] duplicated across partition halves
    KT_ = Kf.transpose(0, 2, 1).astype(np.float16)
    kt_all = np.ascontiguousarray(
        np.concatenate([KT_, KT_], axis=1)
    )  # [BHT, 2D, T]
    # V -> [BHT, P, T/P, D]: v_dev[b, p, t, d] = V[b, t*128 + p, d]
    v_all = np.ascontiguousarray(
        Vf.reshape(BHT, T // P, P, D).transpose(0, 2, 1, 3)
    )

    in_maps = []
    for c in range(N_CORES):
        sl = slice(c * BH, (c + 1) * BH)
        in_maps.append(
            {
                "qt": np.ascontiguousarray(qt_all[sl]),
                "kt": np.ascontiguousarray(kt_all[sl]),
                "v": np.ascontiguousarray(v_all[sl]),
            }
        )

    res = None
    last_err = None
    for attempt in range(3):
        try:
            res = run_bass_kernel_spmd(
                nc, in_maps, core_ids=list(range(N_CORES)), trace=TRACE
            )
            break
        except Exception as e:  # transient device/runtime hiccup -> retry
            last_err = e
            import time as _time

            _time.sleep(2.0)
    if res is None:
        raise last_err
    global LAST_RESULTS
    LAST_RESULTS = res

    # Gather: each core returns out [BH, D, T] -> [BHT, T, D] -> [B, H, T, D]
    outs = [res.results[c]["out"] for c in range(N_CORES)]
    out_all = np.concatenate(outs, axis=0)  # [BHT, D, T]
    out = out_all.transpose(0, 2, 1).reshape(B, H, T, D)
    return np.ascontiguousarray(out).astype(np.float32)


# revision 23
# speedup vs baseline: 1.7186x; 1.7186x over previous
"""Trainium2 Bass kernel for nn_DotProductAttention_11433202942822.

Math (per (b, h) pair, T=2048, D=64):
    S = Q @ K^T * (1/sqrt(64))            [T1, T2]
    attn = softmax(S, axis=T1)            <- softmax over the QUERY axis
    out = attn @ V                        [T1, D]

Structure (v4):
  * S^T = K @ Q^T per 128-row k2-tile, with the two q-halves of each tile
    computed CONCURRENTLY in the two PE row-group halves (contraction is
    d=64, so rows 0-63 and 64-127 hold two independent copies of the
    weights; Q^T/K^T are duplicated across both partition halves).
  * The exp+colsum drain of the PSUM score tiles is split across TWO
    engines per-tile ("routing"):
      - ACT route: scalar-engine activation(Exp, accum_out) -> fp16 et
        plus free column sums via the ACT accumulator.
      - DVE route: one-pass Schraudolph exp on the vector engine:
        int16 bits = trunc(s * A + B) computed by tensor_scalar
        (mult+add, fp32 PSUM -> int16 SBUF), bit-identical to fp16
        exp2 values; the int16 tile is bitcast to fp16 for mm2. The
        column sums are a GPSIMD 2-level pairwise-add tree + one small
        DVE reduce, so the vector engine only pays ~1/4 of a reduce.
  * Softmax normalization is folded into V (vp = V * 1/colsum), batched
    per half-head with a broadcast tensor_tensor.
  * mm2 (out^T += vp^T @ et) packs the 64-wide output into both PE
    column-group halves for 2x concurrency (as row-pairing does for mm1).

Sharding: batch*heads = 32 pairs, 4 per core across 8 cores (head/data
parallel, no cross-core communication).
"""

import sys

import numpy as np

if "/opt/trn_rl_repo" not in sys.path:
    sys.path.insert(0, "/opt/trn_rl_repo")

import concourse.tile as tile  # noqa: E402
from concourse import bacc, mybir  # noqa: E402
from concourse.bass_utils import run_bass_kernel_spmd  # noqa: E402

P = 128
D = 64
SCALE = 1.0 / (D ** 0.5)
N_CORES = 8

F32 = mybir.dt.float32
F16 = mybir.dt.float16
I16 = mybir.dt.int16

# Schraudolph fp16 exp: bits16 = trunc(s * A_TS + B_TS) interpreted as fp16
# gives exp2(s*log2e*SCALE + c') = exp(s*SCALE)*2^c. The uniform 2^c factor
# cancels in the softmax normalization; c is tuned for min rel-rms anyway.
LOG2E = 1.4426950408889634
A_TS = SCALE * LOG2E * 1024.0
B_TS = (15.0 - 0.0575) * 1024.0

# Tiles (of 16 per head) drained by the DVE Schraudolph route; the rest
# go to the ACT exp route. Tuned so ACT-busy ~= DVE-busy ~= GPS-busy.
# D-tiles sit early in each half-head so their (longer-latency) colsum
# chain finishes well before the vp boundary.
ROUTE_D = (0, 2, 4, 8, 10, 12)


def build_attention_nc(BH: int, T: int, debug: bool = False):
    """Build the per-core Bass module.

    Inputs (per core):
      qt  [BH, P, T]     fp16  Q^T duplicated across both partition halves
      kt  [BH, P, T]     fp16  K^T duplicated across both partition halves
      v   [BH, P, T/P, D] f32  V with k2 split (tile, partition)
    Output:
      out [BH, D, T]   f32   out transposed (d-major)
    """
    KT = T // P          # k2 tiles per head (16)
    GRP = 8              # tiles per vp/reciprocal batch
    QH = T // 2          # q half (1024)

    nc = bacc.Bacc("TRN2", target_bir_lowering=False, debug=debug)

    qt = nc.dram_tensor("qt", [BH, P, T], F16, kind="ExternalInput").ap()
    kt = nc.dram_tensor("kt", [BH, P, T], F16, kind="ExternalInput").ap()
    v = nc.dram_tensor("v", [BH, P, KT, D], F32, kind="ExternalInput").ap()
    out = nc.dram_tensor("out", [BH, D, T], F32, kind="ExternalOutput").ap()

    with tile.TileContext(nc) as tc:
        with (
            tc.tile_pool(name="ins", bufs=1) as ins_pool,
            tc.tile_pool(name="et", bufs=22) as et_pool,
            tc.tile_pool(name="gh", bufs=3) as gh_pool,
            tc.tile_pool(name="small", bufs=4) as small_pool,
            tc.tile_pool(name="vps", bufs=4) as vp_pool,
            tc.tile_pool(name="osb", bufs=2) as osb_pool,
            tc.tile_pool(name="spsum", bufs=3, space="PSUM") as s_pool,
            tc.tile_pool(name="opsum", bufs=1, space="PSUM") as o_pool,
        ):
            qt_sb = ins_pool.tile([P, BH, T], F16, tag="qt_sb")
            kt_sb = ins_pool.tile([P, BH, T], F16, tag="kt_sb")
            v_sb = ins_pool.tile([P, BH, KT, D], F32, tag="v_sb")
            zf = ins_pool.tile([P, 512], F16, tag="zf")
            nc.vector.memset(zf[:], 0.0)
            # Warm up the ACT exp table-set during the input DMAs so the
            # first real ACTIVATE doesn't pay the ~2.6us ACT_TABLE_LOAD.
            warm = small_pool.tile([P, 1], F32, tag="warm")
            nc.vector.memset(warm[:], 0.0)
            nc.scalar.activation(
                warm[:], warm[:], mybir.ActivationFunctionType.Exp
            )

            for bh in range(BH):
                if bh == 0:
                    nc.sync.dma_start(kt_sb[:, bh, :], kt[bh])
                    nc.sync.dma_start(qt_sb[:, bh, 0:QH], qt[bh][:, 0:QH])
                    nc.sync.dma_start(qt_sb[:, bh, QH:T], qt[bh][:, QH:T])
                    nc.sync.dma_start(v_sb[:, bh, 0:KT // 2], v[bh][:, 0:KT // 2])
                    nc.sync.dma_start(v_sb[:, bh, KT // 2:KT], v[bh][:, KT // 2:KT])
                else:
                    nc.sync.dma_start(qt_sb[:, bh, :], qt[bh])
                    nc.sync.dma_start(kt_sb[:, bh, :], kt[bh])
                    nc.sync.dma_start(v_sb[:, bh], v[bh])

            # mm2 jobs are emitted lazily (global across bh) so the PE keeps
            # busy while ACT/DVE drain the next tiles' scores.
            pending_mm2 = []
            mm2_started = set()

            def emit_fillers(out_ps_, n):
                # Mathematically no-op matmuls (both operands zero, start
                # =False accumulate) that keep the PE array streaming during
                # the drain-paced mm1 phase, so the HAM clock-gate doesn't
                # re-throttle the PE to 1.2 GHz between mm2 bursts.
                for _ in range(n):
                    nc.tensor.matmul(
                        out_ps_[0:D, 0:128],
                        lhsT=zf[:, 0:D],
                        rhs=zf[:, 0:128],
                        start=False,
                        stop=False,
                        skip_group_check=True,
                    )

            def emit_mm2(job):
                bh_, t_, out_ps_, vp_, ti_, et_ = job
                mm2_started.add(id(out_ps_))
                lhs = vp_[:, ti_, :]
                for qh in (0, 512):
                    nc.tensor.matmul(
                        out_ps_[0:D, qh:qh + 512],
                        lhsT=lhs,
                        rhs=et_[:, qh:qh + 512],
                        start=(t_ == 0),
                        stop=(t_ == KT - 1),
                        skip_group_check=True,
                    )
                    nc.tensor.matmul(
                        out_ps_[D:2 * D, qh:qh + 512],
                        lhsT=lhs,
                        rhs=et_[:, QH + qh:QH + qh + 512],
                        start=(t_ == 0),
                        stop=(t_ == KT - 1),
                        skip_group_check=True,
                    )
                if t_ == KT - 1:
                    # Evacuate out^T: alternate the copy engine per head to
                    # balance ACT/DVE load, then DMA to DRAM.
                    osb = osb_pool.tile([2 * D, QH], F32, tag="osb")
                    nc.scalar.copy(osb[:], out_ps_[:])
                    nc.sync.dma_start(out[bh_][:, 0:QH], osb[0:D])
                    nc.sync.dma_start(out[bh_][:, QH:T], osb[D:2 * D])

            for bh in range(BH):
                out_ps = o_pool.tile([2 * D, QH], F32, tag="out_ps")
                cs0 = small_pool.tile([P, KT], F32, tag="cs0", name="cs0")
                cs1 = small_pool.tile([P, KT], F32, tag="cs1", name="cs1")
                # DVE-route tiles produce a single colsum (into cs0); zero
                # cs1 so the uniform cs0+cs1 add per half-head is correct.
                nc.vector.memset(cs1[:], 0.0)
                et_tiles = {}

                for t in range(KT):
                    spA = s_pool.tile([P, QH], F32, tag="sp", name="spA")
                    spB = s_pool.tile([P, QH], F32, tag="sp", name="spB")
                    lhsA = kt_sb[0:D, bh, t * P:(t + 1) * P]
                    lhsB = kt_sb[D:2 * D, bh, t * P:(t + 1) * P]
                    # q-halves of tile t run concurrently in the two PE
                    # row-group halves.
                    for c in (0, 512):
                        nc.tensor.matmul(
                            spA[:, c:c + 512],
                            lhsT=lhsA,
                            rhs=qt_sb[0:D, bh, c:c + 512],
                            start=True,
                            stop=True,
                        )
                        nc.tensor.matmul(
                            spB[:, c:c + 512],
                            lhsT=lhsB,
                            rhs=qt_sb[D:2 * D, bh, QH + c:QH + c + 512],
                            start=True,
                            stop=True,
                        )

                    et = et_pool.tile([P, T], F16, tag="et")
                    if t in ROUTE_D:
                        # Schraudolph exp on DVE: int16 bits -> fp16 view.
                        et_i = et[:].bitcast(I16)
                        nc.vector.tensor_scalar(
                            et_i[:, 0:QH], spA[:], A_TS, B_TS,
                            mybir.AluOpType.mult, mybir.AluOpType.add,
                        )
                        nc.vector.tensor_scalar(
                            et_i[:, QH:T], spB[:], A_TS, B_TS,
                            mybir.AluOpType.mult, mybir.AluOpType.add,
                        )
                        # 1-level GPSIMD pairwise add halves the DVE reduce.
                        gh = gh_pool.tile([P, QH], F16, tag="gh")
                        nc.gpsimd.tensor_tensor(
                            gh[:], et[:, 0:QH], et[:, QH:T],
                            mybir.AluOpType.add,
                        )
                        nc.vector.tensor_reduce(
                            cs0[:, t:t + 1], gh[:],
                            mybir.AxisListType.X, mybir.AluOpType.add,
                        )
                    else:
                        nc.scalar.activation(
                            et[:, 0:QH], spA[:],
                            mybir.ActivationFunctionType.Exp,
                            scale=SCALE,
                            accum_out=cs0[:, t:t + 1],
                        )
                        nc.scalar.activation(
                            et[:, QH:T], spB[:],
                            mybir.ActivationFunctionType.Exp,
                            scale=SCALE,
                            accum_out=cs1[:, t:t + 1],
                        )

                    et_tiles[t] = et

                    if t % GRP == GRP - 1:
                        # Group boundary: batched reciprocal + V-prescale.
                        # High priority so the DVE runs this promptly and
                        # mm2 (and thus the PE) isn't starved.
                        h0 = t - (GRP - 1)
                        with tc.high_priority():
                            cst = small_pool.tile([P, GRP], F32, tag="cst")
                            nc.gpsimd.tensor_tensor(
                                cst[:], cs0[:, h0:h0 + GRP],
                                cs1[:, h0:h0 + GRP], mybir.AluOpType.add,
                            )
                            rec = small_pool.tile([P, GRP], F32, tag="rec")
                            nc.vector.reciprocal(rec[:], cst[:])
                            vp = vp_pool.tile([P, GRP, D], F16, tag="vp")
                            nc.vector.tensor_tensor(
                                vp[:],
                                rec[:].unsqueeze(2).broadcast_to([P, GRP, D]),
                                v_sb[:, bh, h0:h0 + GRP, :],
                                mybir.AluOpType.mult,
                            )
                        for ti in range(GRP):
                            pending_mm2.append(
                                (bh, h0 + ti, out_ps, vp, ti,
                                 et_tiles[h0 + ti])
                            )

                    # Pop mm2 gradually (a big burst would monopolize the
                    # in-order PE queue and starve mm1 -> ACT/DVE run dry).
                    # Light no-op fillers keep the PE array streaming so the
                    # HAM clock-gate holds it at 2.4 GHz.
                    last = bh == BH - 1 and t == KT - 1
                    if last:
                        while pending_mm2:
                            emit_mm2(pending_mm2.pop(0))
                    else:
                        popped = 0
                        while len(pending_mm2) > 2 and popped < 2:
                            emit_mm2(pending_mm2.pop(0))
                            popped += 1
                        if id(out_ps) in mm2_started:
                            emit_fillers(out_ps, 3)
            while pending_mm2:
                emit_mm2(pending_mm2.pop(0))

    nc.compile()
    return nc


_NC_CACHE: dict = {}

# Debug/profiling knobs (used by the local test harness; harmless defaults).
TRACE = False
LAST_RESULTS = None


def _get_nc(BH: int, T: int):
    key = (BH, T)
    if key not in _NC_CACHE:
        _NC_CACHE[key] = build_attention_nc(BH, T)
    return _NC_CACHE[key]


def _reference_numpy(Q, K, V, padding_mask, isCausal):
    """Fallback exactly mirroring reference.py (never hit for spec inputs)."""
    Q = Q.astype(np.float64)
    K = K.astype(np.float64)
    V = V.astype(np.float64)
    scores = np.einsum("bhqd,bhkd->bhqk", Q, K) * SCALE
    T1 = scores.shape[2]
    mask = padding_mask[:, None, :, :].astype(np.float64)
    if isCausal:
        mask = mask * np.tril(np.ones((T1, T1)))
    scores = np.where(mask == 0, -np.inf, scores)
    m = np.max(scores, axis=2, keepdims=True)
    e = np.exp(scores - m)
    attn = e / np.sum(e, axis=2, keepdims=True)
    return np.einsum("bhqk,bhkd->bhqd", attn, V).astype(np.float32)


def kernel(Q, K, V, padding_mask, isCausal, **_unused):
    Q = np.asarray(Q, dtype=np.float32)
    K = np.asarray(K, dtype=np.float32)
    V = np.asarray(V, dtype=np.float32)
    padding_mask = np.asarray(padding_mask)
    causal = int(np.asarray(isCausal))

    B, H, T, Dd = Q.shape
    assert Dd == D
    if causal != 0 or padding_mask.min() != 1.0 or padding_mask.max() != 1.0:
        return _reference_numpy(Q, K, V, padding_mask, causal)

    BHT = B * H
    assert BHT % N_CORES == 0
    BH = BHT // N_CORES  # pairs per core

    nc = _get_nc(BH, T)

    # Host-side layout prep (contiguous per-core shards).
    Qf = Q.reshape(BHT, T, D)
    Kf = K.reshape(BHT, T, D)
    Vf = V.reshape(BHT, T, D)

    QT = Qf.transpose(0, 2, 1).astype(np.float16)  # [BHT, D, T]
    qt_all = np.ascontiguousarray(
        np.concatenate([QT, QT], axis=1)
    )  # [BHT, 2D, T] duplicated across partition halves
    KT_ = Kf.transpose(0, 2, 1).astype(np.float16)
    kt_all = np.ascontiguousarray(
        np.concatenate([KT_, KT_], axis=1)
    )  # [BHT, 2D, T]
    # V -> [BHT, P, T/P, D]: v_dev[b, p, t, d] = V[b, t*128 + p, d]
    v_all = np.ascontiguousarray(
        Vf.reshape(BHT, T // P, P, D).transpose(0, 2, 1, 3)
    )

    in_maps = []
    for c in range(N_CORES):
        sl = slice(c * BH, (c + 1) * BH)
        in_maps.append(
            {
                "qt": np.ascontiguousarray(qt_all[sl]),
                "kt": np.ascontiguousarray(kt_all[sl]),
                "v": np.ascontiguousarray(v_all[sl]),
            }
        )

    res = None
    last_err = None
    for attempt in range(3):
        try:
            res = run_bass_kernel_spmd(
                nc, in_maps, core_ids=list(range(N_CORES)), trace=TRACE
            )
            break
        except Exception as e:  # transient device/runtime hiccup -> retry
            last_err = e
            import time as _time

            _time.sleep(2.0)
    if res is None:
        raise last_err
    global LAST_RESULTS
    LAST_RESULTS = res

    # Gather: each core returns out [BH, D, T] -> [BHT, T, D] -> [B, H, T, D]
    outs = [res.results[c]["out"] for c in range(N_CORES)]
    out_all = np.concatenate(outs, axis=0)  # [BHT, D, T]
    out = out_all.transpose(0, 2, 1).reshape(B, H, T, D)
    return np.ascontiguousarray(out).astype(np.float32)


# revision 27
# speedup vs baseline: 1.7405x; 1.0128x over previous
"""Trainium2 Bass kernel for nn_DotProductAttention_11433202942822.

Math (per (b, h) pair, T=2048, D=64):
    S = Q @ K^T * (1/sqrt(64))            [T1, T2]
    attn = softmax(S, axis=T1)            <- softmax over the QUERY axis
    out = attn @ V                        [T1, D]

Key restructuring for TRN2:
  * Compute S^T = K @ Q^T with k2 on partitions and q on the free axis, so
    the softmax reduction (over q) is a free-axis reduction that the scalar
    engine produces for free via activation(Exp, accum_out=...).
  * Fold the softmax normalization into V instead of the attention matrix:
        out^T[d, q] = sum_k2 (V[k2, d] / s[k2]) * E^T[k2, q]
    which scales 2048x64 elements instead of 2048x2048.
  * Matmuls run in fp16 (PE upconverts to FP22, accumulates fp32; 1 col/cycle
    at 2.4GHz with fast-weight-load) with N=512 moving chunks.
  * Layout transforms (Q/K transposes, output transpose back to [t, d]) are
    done host-side so every DMA is contiguous.

Sharding: batch*heads = 32 pairs, 4 per core across 8 cores (head/data
parallel, no cross-core communication).
"""

import sys

import numpy as np

if "/opt/trn_rl_repo" not in sys.path:
    sys.path.insert(0, "/opt/trn_rl_repo")

import concourse.tile as tile  # noqa: E402
from concourse import bacc, mybir  # noqa: E402
from concourse.bass_utils import run_bass_kernel_spmd  # noqa: E402

P = 128
D = 64
SCALE = 1.0 / (D ** 0.5)
N_CORES = 8

F32 = mybir.dt.float32
F16 = mybir.dt.float16
I16 = mybir.dt.int16

# Schraudolph fp16 exp for the DVE-offloaded tiles: int16 bits =
# trunc(s * A_TS + B_TS) bit-cast to fp16 equals exp(s*SCALE) * 2^c with a
# uniform 2^c factor that cancels in the softmax normalization (rel-rms
# ~1.7% on the offloaded tiles only).
LOG2E = 1.4426950408889634
A_TS = SCALE * LOG2E * 1024.0
B_TS = (15.0 - 0.0575) * 1024.0

# Tile-pair indices j (of KT_TILES//2 per head) whose ODD tile (tB) is
# drained by the vector engine instead of ScalarE. Keeps the ACT rhythm
# intact while trimming ~13% of its work.
DVE_J = (1, 5)


def build_attention_nc(BH: int, T: int, debug: bool = False):
    """Build the per-core Bass module.

    Inputs (per core):
      qt  [BH, D, T]   fp16  Q transposed (d-major)
      kt  [BH, D, T]   fp16  K transposed (d-major)
      v   [BH, P, T/P, D] f32  V with k2 split (tile, partition)
    Output:
      out [BH, D, T]   f32   out transposed (d-major)
    """
    assert T % 1024 == 0 and T % P == 0
    KT_TILES = T // P  # number of 128-row k2 tiles
    ACT_CHUNK = 1024   # elements per activation instruction (2 PSUM banks)

    nc = bacc.Bacc("TRN2", target_bir_lowering=False, debug=debug)

    # qt: Q^T duplicated on both partition halves; kt: even k2-tiles on
    # partitions 0-63, odd on 64-127 -> mm1 runs tile-pairs concurrently in
    # PE row-groups and LDWEIGHTS pulls ahead (different row_grp).
    qt = nc.dram_tensor("qt", [BH, 2 * D, T], F16, kind="ExternalInput").ap()
    kt = nc.dram_tensor("kt", [BH, 2 * D, T // 2], F16, kind="ExternalInput").ap()
    v = nc.dram_tensor("v", [BH, P, T // P, D], F32, kind="ExternalInput").ap()
    out = nc.dram_tensor("out", [BH, D, T], F32, kind="ExternalOutput").ap()

    with tile.TileContext(nc) as tc:
        with (
            tc.tile_pool(name="ins", bufs=1) as ins_pool,
            tc.tile_pool(name="et", bufs=4) as et_pool,
            tc.tile_pool(name="gh", bufs=2) as gh_pool,
            tc.tile_pool(name="small", bufs=8) as small_pool,
            tc.tile_pool(name="osb", bufs=2) as osb_pool,
            tc.tile_pool(name="spsum", bufs=3, space="PSUM") as s_pool,
            tc.tile_pool(name="opsum", bufs=1, space="PSUM") as o_pool,
        ):
            qt_sb = ins_pool.tile([2 * D, BH, T], F16, tag="qt_sb")
            kt_sb = ins_pool.tile([2 * D, BH, T // 2], F16, tag="kt_sb")
            v_sb = ins_pool.tile([P, BH, KT_TILES, D], F32, tag="v_sb")
            # Warm up the ACT exp table-set during the input DMAs so the
            # first real ACTIVATE doesn't pay the ~2.6us ACT_TABLE_LOAD.
            warm = small_pool.tile([P, 1], F32, tag="warm")
            nc.vector.memset(warm[:], 0.0)
            nc.scalar.activation(
                warm[:], warm[:], mybir.ActivationFunctionType.Exp
            )

            # Per-bh DMA split so the first tile's compute starts as soon as
            # its own slices land. For bh0, land kt first and qt/v in halves
            # so tile-pair 0's matmuls start ~2-3us earlier.
            for bh in range(BH):
                if bh == 0:
                    nc.sync.dma_start(kt_sb[:, bh, :], kt[bh])
                    nc.sync.dma_start(qt_sb[:, bh, 0:T // 2],
                                      qt[bh][:, 0:T // 2])
                    nc.sync.dma_start(qt_sb[:, bh, T // 2:T],
                                      qt[bh][:, T // 2:T])
                    nc.sync.dma_start(v_sb[:, bh, 0:KT_TILES // 2],
                                      v[bh][:, 0:KT_TILES // 2])
                    nc.sync.dma_start(v_sb[:, bh, KT_TILES // 2:KT_TILES],
                                      v[bh][:, KT_TILES // 2:KT_TILES])
                else:
                    nc.sync.dma_start(qt_sb[:, bh, :], qt[bh])
                    nc.sync.dma_start(kt_sb[:, bh, :], kt[bh])
                    nc.sync.dma_start(v_sb[:, bh], v[bh])

            def emit_mm2(out_ps, vp, et, t):
                for c in range(0, T, 512):
                    half = c // (T // 2)  # 0 or 1 -> partition col-group
                    qh = c % (T // 2)
                    nc.tensor.matmul(
                        out_ps[half * D:(half + 1) * D, qh:qh + 512],
                        lhsT=vp[:],
                        rhs=et[:, c:c + 512],
                        start=(t == 0),
                        stop=(t == KT_TILES - 1),
                        # The sim's psum group tracker is partition-base
                        # blind; the two col-groups accumulate disjoint
                        # partition rows of the same banks.
                        skip_group_check=True,
                    )

            def emit_sums_vp(bh, t, et, partial_sums):
                while len(partial_sums) > 1:
                    stot = small_pool.tile([P, 1], F32, tag="stot")
                    nc.vector.tensor_add(
                        stot[:], partial_sums[0][:], partial_sums[1][:]
                    )
                    partial_sums = [stot] + partial_sums[2:]
                rec = small_pool.tile([P, 1], F32, tag="rec")
                nc.vector.reciprocal(rec[:], partial_sums[0][:])
                vp = small_pool.tile([P, D], F16, tag="vp")
                nc.vector.tensor_scalar_mul(vp[:], v_sb[:, bh, t, :], rec[:])
                return vp

            def evacuate(bh, out_ps):
                osb = osb_pool.tile([2 * D, T // 2], F32, tag="osb")
                nc.vector.tensor_copy(osb[:], out_ps[:])
                nc.sync.dma_start(out[bh][:, 0:T // 2], osb[0:D])
                nc.sync.dma_start(out[bh][:, T // 2:T], osb[D:2 * D])

            def pop_mm2(pending):
                bh_, out_ps_, vp_, et_, t_ = pending.pop(0)
                emit_mm2(out_ps_, vp_, et_, t_)
                if t_ == KT_TILES - 1:
                    evacuate(bh_, out_ps_)

            # mm2 pipeline is global across bh boundaries so the next bh's
            # mm1/exp stream keeps ScalarE fed while the previous bh's tail
            # mm2s and evacuation drain.
            pending_mm2 = []
            for bh in range(BH):
                # out^T packed on partitions: rows 0-63 hold d x q[0:T/2],
                # rows 64-127 hold d x q[T/2:T]  -> only T/1024 PSUM banks.
                out_ps = o_pool.tile([2 * D, T // 2], F32, tag="out_ps")
                for j in range(KT_TILES // 2):
                    tA, tB = 2 * j, 2 * j + 1
                    etA = et_pool.tile([P, T], F16, tag="et", name="etA")
                    etB = et_pool.tile([P, T], F16, tag="et", name="etB")
                    lhsA = kt_sb[0:D, bh, j * P:(j + 1) * P]
                    lhsB = kt_sb[D:2 * D, bh, j * P:(j + 1) * P]
                    sps = []
                    for q0 in range(0, T, ACT_CHUNK):
                        spA = s_pool.tile([P, ACT_CHUNK], F32, tag="sp",
                                          name="spA")
                        spB = s_pool.tile([P, ACT_CHUNK], F32, tag="sp",
                                          name="spB")
                        for c in range(0, ACT_CHUNK, 512):
                            nc.tensor.matmul(
                                spA[:, c:c + 512],
                                lhsT=lhsA,
                                rhs=qt_sb[0:D, bh, q0 + c:q0 + c + 512],
                                start=True,
                                stop=True,
                            )
                            nc.tensor.matmul(
                                spB[:, c:c + 512],
                                lhsT=lhsB,
                                rhs=qt_sb[D:2 * D, bh, q0 + c:q0 + c + 512],
                                start=True,
                                stop=True,
                            )
                        sps.append((q0, spA, spB))
                    for which, et in ((1, etA), (2, etB)):
                        partial_sums = []
                        if which == 2 and j in DVE_J:
                            # DVE-drained tile: one-pass Schraudolph exp
                            # (fp32 PSUM -> int16 bits == fp16 exp values),
                            # colsum via a GPSIMD pairwise add + DVE reduce.
                            et_i = et[:].bitcast(I16)
                            for q0, spA, spB in sps:
                                nc.vector.tensor_scalar(
                                    et_i[:, q0:q0 + ACT_CHUNK],
                                    spB[:], A_TS, B_TS,
                                    mybir.AluOpType.mult,
                                    mybir.AluOpType.add,
                                )
                            gh = gh_pool.tile([P, T // 2], F16, tag="gh")
                            nc.gpsimd.tensor_tensor(
                                gh[:], et[:, 0:T // 2], et[:, T // 2:T],
                                mybir.AluOpType.add,
                            )
                            acc = small_pool.tile([P, 1], F32, tag="acc")
                            nc.vector.tensor_reduce(
                                acc[:], gh[:],
                                mybir.AxisListType.X, mybir.AluOpType.add,
                            )
                            partial_sums.append(acc)
                        else:
                            for q0, spA, spB in sps:
                                sp = spA if which == 1 else spB
                                acc = small_pool.tile([P, 1], F32, tag="acc")
                                nc.scalar.activation(
                                    et[:, q0:q0 + ACT_CHUNK],
                                    sp[:],
                                    mybir.ActivationFunctionType.Exp,
                                    scale=SCALE,
                                    accum_out=acc[:],
                                )
                                partial_sums.append(acc)
                        t = tA if which == 1 else tB
                        vp = emit_sums_vp(bh, t, et, partial_sums)
                        pending_mm2.append((bh, out_ps, vp, et, t))
                    limit = (0 if (bh == BH - 1 and j == KT_TILES // 2 - 1)
                             else 2)
                    while len(pending_mm2) > limit:
                        pop_mm2(pending_mm2)
            while pending_mm2:
                pop_mm2(pending_mm2)

    nc.compile()
    return nc


_NC_CACHE: dict = {}

# Debug/profiling knobs (used by the local test harness; harmless defaults).
TRACE = False
LAST_RESULTS = None


def _get_nc(BH: int, T: int):
    key = (BH, T)
    if key not in _NC_CACHE:
        _NC_CACHE[key] = build_attention_nc(BH, T)
    return _NC_CACHE[key]


def _reference_numpy(Q, K, V, padding_mask, isCausal):
    """Fallback exactly mirroring reference.py (never hit for spec inputs)."""
    Q = Q.astype(np.float64)
    K = K.astype(np.float64)
    V = V.astype(np.float64)
    scores = np.einsum("bhqd,bhkd->bhqk", Q, K) * SCALE
    T1 = scores.shape[2]
    mask = padding_mask[:, None, :, :].astype(np.float64)
    if isCausal:
        mask = mask * np.tril(np.ones((T1, T1)))
    scores = np.where(mask == 0, -np.inf, scores)
    m = np.max(scores, axis=2, keepdims=True)
    e = np.exp(scores - m)
    attn = e / np.sum(e, axis=2, keepdims=True)
    return np.einsum("bhqk,bhkd->bhqd", attn, V).astype(np.float32)


def kernel(Q, K, V, padding_mask, isCausal, **_unused):
    Q = np.asarray(Q, dtype=np.float32)
    K = np.asarray(K, dtype=np.float32)
    V = np.asarray(V, dtype=np.float32)
    padding_mask = np.asarray(padding_mask)
    causal = int(np.asarray(isCausal))

    B, H, T, Dd = Q.shape
    assert Dd == D
    if causal != 0 or padding_mask.min() != 1.0 or padding_mask.max() != 1.0:
        return _reference_numpy(Q, K, V, padding_mask, causal)

    BHT = B * H
    assert BHT % N_CORES == 0
    BH = BHT // N_CORES  # pairs per core

    nc = _get_nc(BH, T)

    # Host-side layout prep (contiguous per-core shards).
    Qf = Q.reshape(BHT, T, D)
    Kf = K.reshape(BHT, T, D)
    Vf = V.reshape(BHT, T, D)

    QT = Qf.transpose(0, 2, 1).astype(np.float16)  # [BHT, D, T]
    qt_all = np.ascontiguousarray(
        np.concatenate([QT, QT], axis=1)
    )  # [BHT, 2D, T] fp16, duplicated across partition halves
    KT = Kf.transpose(0, 2, 1).astype(np.float16)  # [BHT, D, T]
    KT4 = KT.reshape(BHT, D, T // 128, 128)
    kt_all = np.ascontiguousarray(
        np.concatenate(
            [
                KT4[:, :, 0::2, :].reshape(BHT, D, T // 2),
                KT4[:, :, 1::2, :].reshape(BHT, D, T // 2),
            ],
            axis=1,
        )
    )  # [BHT, 2D, T/2] fp16: even k2-tiles top, odd bottom
    # V -> [BHT, P, T/P, D]: v_dev[b, p, t, d] = V[b, t*128 + p, d]
    v_all = np.ascontiguousarray(
        Vf.reshape(BHT, T // P, P, D).transpose(0, 2, 1, 3)
    )

    in_maps = []
    for c in range(N_CORES):
        sl = slice(c * BH, (c + 1) * BH)
        in_maps.append(
            {
                "qt": np.ascontiguousarray(qt_all[sl]),
                "kt": np.ascontiguousarray(kt_all[sl]),
                "v": np.ascontiguousarray(v_all[sl]),
            }
        )

    res = None
    last_err = None
    for attempt in range(3):
        try:
            res = run_bass_kernel_spmd(
                nc, in_maps, core_ids=list(range(N_CORES)), trace=TRACE
            )
            break
        except Exception as e:  # transient device/runtime hiccup -> retry
            last_err = e
            import time as _time

            _time.sleep(2.0)
    if res is None:
        raise last_err
    global LAST_RESULTS
    LAST_RESULTS = res

    # Gather: each core returns out [BH, D, T] -> [BHT, T, D] -> [B, H, T, D]
    outs = [res.results[c]["out"] for c in range(N_CORES)]
    out_all = np.concatenate(outs, axis=0)  # [BHT, D, T]
    out = out_all.transpose(0, 2, 1).reshape(B, H, T, D)
    return np.ascontiguousarray(out).astype(np.float32)

